# revision 7
# baseline (speedup 1.0000x reference)
"""GCN (3-layer, skip-concat) on 8 Trainium2 NeuronCores.

Strategy (hardcoded for N=10000, E=320000, dims 512/512/256):
  - Row-partition nodes across 8 cores (1280 padded rows each, N padded
    to 10240). The adjacency shard is densified on the host into
    A_k^T [10240, 1280] fp8 (e4m3 for layers 0/1 DoubleRow, e3m4 for
    the final layer's plain-rate SpMM).
  - Layer 0 runs SPMM-FIRST: since A@(x@W0) = (A@x)@W0 and x is
    replicated on every core, the kernel skips the full dense x@W0
    entirely. x is staged host-side in the gathered node-major fp8
    layout (same layout the SpMM stationary wants), the L0 SpMM
    computes Z^T = (A_k @ x)^T feature-major, and a tiny dense
    (W0 stationary, 0.34 GMAC) produces Y0^T = W0^T Z^T. No layer-0
    collective at all; the CC-stream bootstrap (~60-75us) hides behind
    the L0 SpMM.
  - Layers 0/1 SpMM: S blocks (e4m3) stationary + A^T (e4m3) moving with
    perf_mode=DoubleRow -> 256-deep contraction per instruction. W0 is
    host-scaled by 16 (clears the e4m3 denormal floor); the Y0 sink
    applies 1/16. Layer 2 SpMM: e3m4 S (host-scale 1/8, sink x8) x e3m4
    A, plain matmuls (DR requires e4m3 and would roughly double the
    output error). End-to-end rel err ~1.1e-2 (gate 2e-2).
  - Activations live feature-major (X^T) in SBUF. Per layer:
      S_k   = X^T.T @ W          (node-major [1280, d_out], PE)
      S     = AllGather(S_k)     (fp8, HBM collective, 3 chunks)
      Y^T   = S_tiles^T @ A_k^T  (feature-major, PE; S tiles stationary)
      X' ^T = [relu(Y^T + b); (Y^T + b)]   (partition-axis concat, free)
  - Chunking: 3 gathered chunks per layer = local row ranges
    [0,512) / [512,1024) / [1024,1280), produced by passes nci0/1/2.
  - A tiny warm-up AllGather at program start absorbs the CC-stream
    bootstrap (~35-110us) + initial cross-core skew, so the first real
    gather never eats them.
  - Queue discipline: sync = x_ch blocks + adjacency prefetch + deferred
    W0/W1/W2/bias loads (trickled); scalar = epilogue drains (alternating
    with vector) + bounce writes + output writes; VECTOR ring = gathered-
    chunk reloads (s_load) so a reload waiting on an AllGather can never
    head-of-line-block the bounce writes that trigger the NEXT AllGather;
    gpsimd = collective triggers.
  - SpMM PSUM tiles rotate through 5 tags so the first matmul of each
    pass never waits on the previous pass's epilogue draining the bank
    (5 spmm banks + 2 dense banks = 7 of 8).
"""

import os
import numpy as np
from ml_dtypes import bfloat16, float8_e3m4, float8_e4m3

N = 10000
NPAD = 10240
NCORES = 8
R = NPAD // NCORES  # 1280 rows per core
P = 128
CT = NPAD // P  # 80 contraction tiles for the SpMM
KSUB = 4  # contraction subtiles per adjacency DMA block

D0_IN, D0_OUT = 512, 512
D1_IN, D1_OUT = 1024, 512
D2_IN, D2_OUT = 1024, 256

# n-chunks of the 1280-wide free dim (PSUM bank = 512 fp32); pass nci
# produces output chunk nci (same index).
N_CHUNKS = [(0, 512), (512, 512), (1024, 256)]
NCI_ORDER = (0, 1, 2)
# chunk c -> (local row0, nrows, gathered ktiles, KSUB-groups)
CHUNKS = {0: (0, 512, 32, 8), 1: (512, 512, 32, 8), 2: (1024, 256, 16, 4)}
C_ORDER = (0, 1, 2)  # contraction + production order
CH_BASE = {0: 0, 1: 4096, 2: 8192}  # gathered-row base of each chunk
# flat adjacency block order: (chunk, group) pairs in contraction order
BLOCKS = [(c, g) for c in C_ORDER for g in range(CHUNKS[c][3])]  # 20
# dense m-tiles whose lhsT columns come from pass nci's epilogue
M_OF_NC = {0: (0, 1, 2, 3), 1: (4, 5, 6, 7), 2: (8, 9)}

# layer-2 SpMM in e4m3 DoubleRow (faster, ~+0.006 rel err) vs e3m4 plain
L2_DR = bool(int(os.environ.get("GCN_L2DR", "0")))

_CACHE = {}
LAST_RESULT = None  # BassKernelResults of the most recent run (for test.py)


def _build_bass():
    import concourse.bass as bass
    import concourse.bacc as bacc
    import concourse.mybir as mybir
    import concourse.tile as tile

    dt = mybir.dt
    bf16 = dt.bfloat16
    e4 = dt.float8e4
    e3 = dt.float8e3
    f32 = dt.float32
    ts = bass.ts
    DR = mybir.MatmulPerfMode.DoubleRow

    nc = bacc.Bacc(
        "TRN2",
        target_bir_lowering=False,
        debug=False,
        enable_asserts=False,
        num_devices=NCORES,
    )

    # x in gathered node-major fp8 blocks (same layout as the SpMM
    # stationary chunks): [20 blocks, P, KSUB, 512]
    xch_d = nc.dram_tensor("xch", [20, P, KSUB, 512], e4, kind="ExternalInput")
    # adjacency blocks [P, KSUB, nw]: e4m3 copy (layers 0/1, DoubleRow)
    # and e3m4 copy (layer 2); nci0/1 are the 512-wide column chunks.
    adjA01_d = nc.dram_tensor("adjA01", [2, 20, P, KSUB, 512], e4, kind="ExternalInput")
    adjA2_d = nc.dram_tensor("adjA2", [20, P, KSUB, 256], e4, kind="ExternalInput")
    if not L2_DR:
        adjB01_d = nc.dram_tensor("adjB01", [2, 20, P, KSUB, 512], e3, kind="ExternalInput")
        adjB2_d = nc.dram_tensor("adjB2", [20, P, KSUB, 256], e3, kind="ExternalInput")
    W_d = [
        nc.dram_tensor("W0", [D0_IN, D0_OUT], e4, kind="ExternalInput"),
        nc.dram_tensor("W1", [D1_IN, D1_OUT], bf16, kind="ExternalInput"),
        nc.dram_tensor("W2", [D2_IN, D2_OUT], bf16, kind="ExternalInput"),
    ]
    b_d = [
        nc.dram_tensor("b0", [D0_OUT, 1], f32, kind="ExternalInput"),
        nc.dram_tensor("b1", [D1_OUT, 1], f32, kind="ExternalInput"),
        nc.dram_tensor("b2", [D2_OUT, 1], f32, kind="ExternalInput"),
    ]
    outT_d = nc.dram_tensor("outT", [D2_OUT, R], f32, kind="ExternalOutput")

    DIMS = [(D0_IN, D0_OUT), (D1_IN, D1_OUT), (D2_IN, D2_OUT)]
    # S dtype on the gather path (stationary dtype of the CONSUMING spmm)
    S2_DT = e4 if L2_DR else e3
    S_DT = [None, e4, S2_DT]
    S2_SCALE = 1.0 if L2_DR else 0.125  # host-semantics scale of S2

    with tile.TileContext(nc) as tc:
        ctx_pools = (
            tc.tile_pool(name="persist", bufs=1),
            tc.tile_pool(name="work", bufs=3),
            tc.tile_pool(name="psum", bufs=1, space="PSUM"),
            tc.tile_pool(name="dram", bufs=1, space="DRAM"),
        )
        with ctx_pools[0] as persist, ctx_pools[1] as work, \
             ctx_pools[2] as psum_pool, ctx_pools[3] as dram_pool:

            # ---- resident weights / biases (all trickled on sync) ----
            w_sb = []
            for L, (d_in, d_out) in enumerate(DIMS):
                wt = persist.tile(
                    [P, d_in // P, d_out], e4 if L == 0 else bf16,
                    name=f"w{L}", tag=f"w{L}"
                )
                w_sb.append(wt)

            b_sb = []
            for L, (d_in, d_out) in enumerate(DIMS):
                tiles = []
                for pi in range(d_out // P):
                    bt = persist.tile([P, 1], f32, name=f"b{L}_{pi}", tag=f"b{L}_{pi}")
                    tiles.append(bt)
                b_sb.append(tiles)

            def deferred_weight_dmas():
                """generator of thunks: W0/W1/W2/bias loads, trickled into
                the sync queue's spare issue slots during L0's first pass.
                Order matters: w0+b0 are needed first (~45us), W1 next
                (~55us), W2 last (~150us)."""
                for c in range(DIMS[0][0] // P):
                    yield lambda c=c: nc.sync.dma_start(
                        w_sb[0][:, c, :], W_d[0][ts(c, P), :]
                    )
                for pi in range(DIMS[0][1] // P):
                    yield lambda pi=pi: nc.sync.dma_start(
                        b_sb[0][pi][:], b_d[0][ts(pi, P), :]
                    )
                for L in (1, 2):
                    d_in = DIMS[L][0]
                    for c in range(d_in // P):
                        yield lambda L=L, c=c: nc.sync.dma_start(
                            w_sb[L][:, c, :], W_d[L][ts(c, P), :]
                        )
                for L in (1, 2):
                    for pi in range(DIMS[L][1] // P):
                        yield lambda L=L, pi=pi: nc.sync.dma_start(
                            b_sb[L][pi][:], b_d[L][ts(pi, P), :]
                        )

            # ---- activations X^T (feature-major), one 3D tile per layer ----
            xt1 = persist.tile([P, D1_IN // P, R], bf16, name="xt1", tag="xt1")
            xt2 = persist.tile([P, D2_IN // P, R], bf16, name="xt2", tag="xt2")
            xts = [None, xt1, xt2]

            # Z^T = (A_k @ x)^T feature-major, fp8 (rhs of the W0 dense)
            zt = persist.tile([P, D0_IN // P, R], e4, name="zt", tag="zt")

            # gathered stationary chunks: s_ch8 holds x (layer 0) then S1
            # (layer 1); s_ch16 holds S2 for the final layer.
            s_ch8 = {
                c: persist.tile(
                    [P, CHUNKS[c][2], 512], e4, name=f"s8_{c}", tag=f"s8_{c}"
                )
                for c in range(3)
            }
            s_ch16 = {
                c: persist.tile(
                    [P, CHUNKS[c][2], 256], S2_DT, name=f"s16_{c}", tag=f"s16_{c}"
                )
                for c in range(3)
            }
            s_ch_of = [s_ch8, s_ch8, s_ch16]

            # tiny warm-up collective: absorbs the CC-stream bootstrap
            # (~35-110us, starts ~21us in) and initial cross-core skew so
            # the first REAL all-gather doesn't eat them.
            cc_warm_in = dram_pool.tile([8, 8], bf16, name="ccw_in", tag="ccw_in")
            cc_warm_out = dram_pool.tile(
                [8 * NCORES, 8], bf16, name="ccw_out", tag="ccw_out",
                addr_space="Shared",
            )

            s_bounce = [
                None,
                dram_pool.tile([R, DIMS[1][1]], e4, name="s_bounce1", tag="sb1"),
                dram_pool.tile([R, DIMS[2][1]], S2_DT, name="s_bounce2", tag="sb2"),
            ]
            s_all = [
                None,
                {
                    c: dram_pool.tile(
                        [NCORES * CHUNKS[c][1], DIMS[1][1]], e4,
                        name=f"s_all1_{c}", tag=f"sa1_{c}", addr_space="Shared",
                    )
                    for c in range(3)
                },
                {
                    c: dram_pool.tile(
                        [NCORES * CHUNKS[c][1], DIMS[2][1]], S2_DT,
                        name=f"s_all2_{c}", tag=f"sa2_{c}", addr_space="Shared",
                    )
                    for c in range(3)
                },
            ]

            # rotating spmm psum tags: 5 tags over passes of <=4 tiles each
            sp_ctr = [0]

            def dense_m(L, m):
                """dense S_k m-tile: psum = xt.T @ W, cast to S dtype, bounce."""
                d_in, d_out = DIMS[L]
                n_ct = d_in // P
                dps = psum_pool.tile(
                    [P, d_out], f32, name=f"dps_{L}_{m}", tag="dense_ps", bufs=2
                )
                for c in range(n_ct):
                    nc.tensor.matmul(
                        dps[:],
                        lhsT=xts[L][:, c, ts(m, P)],
                        rhs=w_sb[L][:, c, :],
                        start=(c == 0),
                        stop=(c == n_ct - 1),
                    )
                s_sb = work.tile(
                    [P, d_out], S_DT[L], name=f"ssb_{L}_{m}", tag=f"s_sb{L}", bufs=4
                )
                # alternate the psum->fp8 drain between vector and
                # scalar so neither engine rate-limits the dense bursts
                sc = S2_SCALE if L == 2 else 1.0
                if m % 2 == 0:
                    if sc == 1.0:
                        nc.vector.tensor_copy(s_sb[:], dps[:])
                    else:
                        nc.vector.tensor_scalar_mul(s_sb[:], dps[:], sc)
                else:
                    nc.scalar.activation(
                        s_sb[:], dps[:], mybir.ActivationFunctionType.Copy,
                        scale=sc,
                    )
                # bounce on the sync queue: the adjacency prefetch there has
                # many buffers of slack, while the scalar ring carries the
                # s_load reloads which stall on AllGather completion (a
                # bounce stuck behind one would delay the NEXT AllGather).
                nc.sync.dma_start(s_bounce[L][ts(m, P), :], s_sb[:])

            def cc_warmup():
                nc.gpsimd.collective_compute(
                    "AllGather",
                    mybir.AluOpType.bypass,
                    replica_groups=[list(range(NCORES))],
                    ins=[cc_warm_in[:].opt()],
                    outs=[cc_warm_out[:].opt()],
                )

            def ag_issue(L, c):
                """all-gather chunk c of layer L's S (writes s_all only)."""
                r0, nrows, _, _ = CHUNKS[c]
                nc.gpsimd.collective_compute(
                    "AllGather",
                    mybir.AluOpType.bypass,
                    replica_groups=[list(range(NCORES))],
                    ins=[s_bounce[L][r0 : r0 + nrows, :].opt()],
                    outs=[s_all[L][c].opt()],
                )

            def s_load(L, c):
                """load gathered chunk c into SBUF for layer L's spmm, in
                two halves so the first k-tiles land sooner. Issued on the
                scalar ring, which carries ONLY these reloads (+ late output
                writes): a reload stuck waiting on an AllGather can then
                never block the bounce writes (sync ring) that trigger the
                next AllGather."""
                d_out = DIMS[L][1]
                src = s_all[L][c].rearrange("(t p) d -> p t d", p=P)
                dst = s_ch_of[L][c]
                kt = CHUNKS[c][2]
                h = kt // 2
                nc.scalar.dma_start(dst[:, :h, :d_out], src[:, :h, :])
                nc.scalar.dma_start(dst[:, h:, :d_out], src[:, h:, :])

            def spmm_pass_dr(L, nci, sink, chunk_hook=None, n_po=None,
                            block_hook=None):
                """DoubleRow SpMM pass: e4m3 stationary x e4m3 A moving.

                chunk_hook(c) is emitted right after chunk c's last block in
                the contraction loop -- used on the layer's final pass to
                emit the next layer's s_ch reloads as early as the WAR
                hazard allows (ahead of this pass's sinks in queue order).
                block_hook(bi) is emitted just BEFORE block bi's adjacency
                DMA -- layer 0's first pass uses it to interleave the x_ch
                stationary loads with the adjacency stream.
                """
                if n_po is None:
                    n_po = DIMS[L][1] // P
                s_ch = s_ch_of[L]
                n0, nw = N_CHUNKS[nci]
                sp_ps = []
                for p in range(n_po):
                    tag = sp_ctr[0] % 5
                    sp_ctr[0] += 1
                    sp_ps.append(
                        psum_pool.tile(
                            [P, nw], f32, name=f"sp_{L}_{nci}_{p}", tag=f"sp{tag}"
                        )
                    )
                first = True
                for bi, (c, g) in enumerate(BLOCKS):
                    if block_hook is not None:
                        block_hook(bi)
                    if nci < 2:
                        at = work.tile(
                            [P, KSUB, 512], e4,
                            name=f"a4_{L}_{nci}_{bi}", tag="at4", bufs=12,
                        )
                        nc.sync.dma_start(at[:], adjA01_d[nci, bi])
                    else:
                        at = work.tile(
                            [P, KSUB, 256], e4,
                            name=f"a4n2_{L}_{bi}", tag="at4n2", bufs=8,
                        )
                        nc.sync.dma_start(at[:], adjA2_d[bi])
                    for sp in range(2):
                        last = (bi == len(BLOCKS) - 1 and sp == 1)
                        kk = g * KSUB + 2 * sp
                        for p in range(n_po):
                            nc.tensor.matmul(
                                sp_ps[p][:],
                                lhsT=s_ch[c][:, kk : kk + 2, ts(p, P)],
                                rhs=at[:, 2 * sp : 2 * sp + 2, :],
                                start=first,
                                stop=last,
                                perf_mode=DR,
                            )
                        first = False
                    if chunk_hook is not None and g == CHUNKS[c][3] - 1:
                        chunk_hook(c)
                for p in range(n_po):
                    sink(p, sp_ps[p], n0, nw)

            def spmm_merged_l2_dr(sink):
                """Merged nci0+nci2 DR pass for layer 2 (e4m3): each
                stationary streams 768 moving columns (512 + 256)."""
                n_po = DIMS[2][1] // P  # 2
                sp_ps = []
                for i in range(2 * n_po):
                    tag = sp_ctr[0] % 5
                    sp_ctr[0] += 1
                    sp_ps.append(
                        psum_pool.tile(
                            [P, 512], f32, name=f"l2m_{i}", tag=f"sp{tag}"
                        )
                    )
                ps0 = sp_ps[:n_po]
                ps2 = sp_ps[n_po:]
                first = True
                for bi, (c, g) in enumerate(BLOCKS):
                    at0 = work.tile(
                        [P, KSUB, 512], e4, name=f"l2a_{bi}", tag="at4", bufs=12,
                    )
                    nc.sync.dma_start(at0[:], adjA01_d[0, bi])
                    at2 = work.tile(
                        [P, KSUB, 256], e4, name=f"l2c_{bi}", tag="at4n2", bufs=8,
                    )
                    nc.sync.dma_start(at2[:], adjA2_d[bi])
                    for sp in range(2):
                        last = (bi == len(BLOCKS) - 1 and sp == 1)
                        kk = g * KSUB + 2 * sp
                        for p in range(n_po):
                            lhsT = s_ch16[c][:, kk : kk + 2, ts(p, P)]
                            nc.tensor.matmul(
                                ps0[p][:, :],
                                lhsT=lhsT,
                                rhs=at0[:, 2 * sp : 2 * sp + 2, :],
                                start=first,
                                stop=last,
                                perf_mode=DR,
                            )
                            nc.tensor.matmul(
                                ps2[p][:, :256],
                                lhsT=lhsT,
                                rhs=at2[:, 2 * sp : 2 * sp + 2, :],
                                start=first,
                                stop=last,
                                perf_mode=DR,
                            )
                        first = False
                for p in range(n_po):
                    sink(p, ps0[p], 0, 512)
                for p in range(n_po):
                    sink(p, ps2[p][:, :256], 1024, 256)

            def spmm_pass_l2(nci, sink):
                """Layer-2 SpMM pass: e3m4 S stationary x e3m4 A moving."""
                n_po = DIMS[2][1] // P
                n0, nw = N_CHUNKS[nci]
                sp_ps = []
                for p in range(n_po):
                    tag = sp_ctr[0] % 5
                    sp_ctr[0] += 1
                    sp_ps.append(
                        psum_pool.tile(
                            [P, nw], f32, name=f"sp_2_{nci}_{p}", tag=f"sp{tag}"
                        )
                    )
                first = True
                for bi, (c, g) in enumerate(BLOCKS):
                    if nci < 2:
                        at = work.tile(
                            [P, KSUB, 512], e3,
                            name=f"a3_{nci}_{bi}", tag="at3", bufs=10,
                        )
                        nc.sync.dma_start(at[:], adjB01_d[nci, bi])
                    else:
                        at = work.tile(
                            [P, KSUB, 256], e3,
                            name=f"a3n2_{bi}", tag="at3n2", bufs=5,
                        )
                        nc.sync.dma_start(at[:], adjB2_d[bi])
                    for s in range(KSUB):
                        last = (bi == len(BLOCKS) - 1 and s == KSUB - 1)
                        for p in range(n_po):
                            nc.tensor.matmul(
                                sp_ps[p][:],
                                lhsT=s_ch16[c][:, g * KSUB + s, ts(p, P)],
                                rhs=at[:, s, :],
                                start=first,
                                stop=last,
                            )
                        first = False
                for p in range(n_po):
                    sink(p, sp_ps[p], n0, nw)

            def spmm_merged_l2(sink):
                """Merged nci0+nci2 pass for layer 2 (plain e3m4 matmuls):
                each stationary LDW streams 768 columns (512 + 256), hiding
                the LDWEIGHTS that otherwise dominate the narrow nci2 pass.
                Each 256-wide output gets its OWN full PSUM bank."""
                n_po = DIMS[2][1] // P  # 2
                sp_ps = []
                for i in range(2 * n_po):
                    tag = sp_ctr[0] % 5
                    sp_ctr[0] += 1
                    sp_ps.append(
                        psum_pool.tile(
                            [P, 512], f32, name=f"l2m_{i}", tag=f"sp{tag}"
                        )
                    )
                ps0 = sp_ps[:n_po]
                ps2 = sp_ps[n_po:]
                first = True
                for bi, (c, g) in enumerate(BLOCKS):
                    at0 = work.tile(
                        [P, KSUB, 512], e3, name=f"l2a_{bi}", tag="at3", bufs=10,
                    )
                    nc.sync.dma_start(at0[:], adjB01_d[0, bi])
                    at2 = work.tile(
                        [P, KSUB, 256], e3, name=f"l2c_{bi}", tag="at3n2", bufs=8,
                    )
                    nc.sync.dma_start(at2[:], adjB2_d[bi])
                    for s in range(KSUB):
                        last = (bi == len(BLOCKS) - 1 and s == KSUB - 1)
                        for p in range(n_po):
                            lhsT = s_ch16[c][:, g * KSUB + s, ts(p, P)]
                            nc.tensor.matmul(
                                ps0[p][:, :],
                                lhsT=lhsT,
                                rhs=at0[:, s, :],
                                start=first,
                                stop=last,
                            )
                            nc.tensor.matmul(
                                ps2[p][:, :256],
                                lhsT=lhsT,
                                rhs=at2[:, s, :],
                                start=first,
                                stop=last,
                            )
                        first = False
                for p in range(n_po):
                    sink(p, ps0[p], 0, 512)
                for p in range(n_po):
                    sink(p, ps2[p][:, :256], 1024, 256)

            def sink_z(p, ps, n0, nw):
                """drain Z psum -> zt fp8 (no scale), alternating engines."""
                if p % 2 == 0:
                    nc.vector.tensor_copy(zt[:, p, n0 : n0 + nw], ps[:])
                else:
                    nc.scalar.activation(
                        zt[:, p, n0 : n0 + nw], ps[:],
                        mybir.ActivationFunctionType.Copy,
                    )

            def w0_dense(nci):
                """Y0^T chunk = W0^T @ Z^T chunk (DR, W0 stationary),
                sinking straight into xt1 (relu+bias / +bias concat)."""
                n0, nw = N_CHUNKS[nci]
                n_po = DIMS[0][1] // P  # 4
                n_ct = DIMS[0][0] // P  # 4
                for po in range(n_po):
                    dps = psum_pool.tile(
                        [P, nw], f32, name=f"w0d_{nci}_{po}", tag="dense_ps",
                        bufs=2,
                    )
                    for sp in range(n_ct // 2):
                        nc.tensor.matmul(
                            dps[:],
                            lhsT=w_sb[0][:, 2 * sp : 2 * sp + 2, ts(po, P)],
                            rhs=zt[:, 2 * sp : 2 * sp + 2, n0 : n0 + nw],
                            start=(sp == 0),
                            stop=(sp == n_ct // 2 - 1),
                            perf_mode=DR,
                        )
                    # psum holds 16*Y0 (W0 host-scaled); sink applies 1/16
                    nc.scalar.activation(
                        xt1[:, po, n0 : n0 + nw],
                        dps[:],
                        mybir.ActivationFunctionType.Relu,
                        bias=b_sb[0][po][:],
                        scale=1.0 / 16.0,
                    )
                    nc.vector.tensor_scalar(
                        xt1[:, n_po + po, n0 : n0 + nw],
                        dps[:], 1.0 / 16.0, b_sb[0][po][:],
                        mybir.AluOpType.mult, mybir.AluOpType.add,
                    )

            def sink_mid(L):
                n_po = DIMS[L][1] // P

                def sink(p, ps, n0, nw):
                    nc.scalar.activation(
                        xts[L + 1][:, p, n0 : n0 + nw],
                        ps[:],
                        mybir.ActivationFunctionType.Relu,
                        bias=b_sb[L][p][:],
                    )
                    nc.vector.tensor_scalar_add(
                        xts[L + 1][:, n_po + p, n0 : n0 + nw],
                        ps[:],
                        b_sb[L][p][:],
                    )

                return sink

            def sink_out(p, ps, n0, nw):
                ot = work.tile([P, nw], f32, name=f"ot_{n0}_{p}", tag="ot", bufs=3)
                nc.vector.tensor_scalar(
                    ot[:], ps[:, :], 1.0 / S2_SCALE, b_sb[2][p][:],
                    mybir.AluOpType.mult, mybir.AluOpType.add,
                )
                nc.scalar.dma_start(outT_d[ts(p, P), n0 : n0 + nw], ot[:])

            # ================= pipeline =================
            cc_warmup()

            wgen = deferred_weight_dmas()

            def xch_block_hook(bi):
                """interleave x_ch stationary loads with pass 0's adjacency
                stream, keeping the x_ch stream 4 blocks AHEAD of the
                adjacency stream (the stationary is the fresh dependency),
                then trickle the resident weight loads once the pipeline
                is primed."""
                if bi == 0:
                    for bj in range(4):
                        c, g = BLOCKS[bj]
                        nc.sync.dma_start(
                            s_ch8[c][:, g * KSUB : (g + 1) * KSUB, :],
                            xch_d[bj],
                        )
                bj = bi + 4
                if bj < len(BLOCKS):
                    c, g = BLOCKS[bj]
                    nc.sync.dma_start(
                        s_ch8[c][:, g * KSUB : (g + 1) * KSUB, :], xch_d[bj]
                    )
                if bi >= 10:
                    for _ in range(2):
                        th = next(wgen, None)
                        if th is not None:
                            th()

            # ---- layer 0: spmm-first ----
            for nci in NCI_ORDER:
                hook = None
                if nci == NCI_ORDER[-1]:
                    # L1's s_ch8 reloads, emitted chunk-by-chunk as the
                    # WAR hazard (this pass's chunk-c reads) clears.
                    hook = lambda c: s_load(1, c) if c != 2 else None
                spmm_pass_dr(
                    0, nci, sink_z, chunk_hook=hook,
                    block_hook=xch_block_hook if nci == 0 else None,
                )
                # trickle remaining weight/bias loads between passes
                for _ in range(8):
                    th = next(wgen, None)
                    if th is not None:
                        th()
                # the W0 dense + next-layer dense + bounce chain feeds this
                # chunk's AllGather trigger: high priority so the scheduler
                # runs it the moment its inputs are ready instead of behind
                # the next spmm pass (the AG chain is the critical path).
                with tc.high_priority():
                    w0_dense(nci)
                    for m in M_OF_NC[nci]:
                        dense_m(1, m)
                ag_issue(1, nci)
                if nci == NCI_ORDER[-1]:
                    s_load(1, 2)

            # ---- layer 1 spmm + layer 2 dense + S2 gathers ----
            for nci in NCI_ORDER:
                spmm_pass_dr(1, nci, sink_mid(1))
                with tc.high_priority():
                    for m in M_OF_NC[nci]:
                        dense_m(2, m)
                ag_issue(2, nci)
                # s_ch16 is untouched before layer 2: load each gathered
                # chunk as soon as its AG is issued (scalar ring, so it
                # can never block the sync-ring bounce writes).
                s_load(2, nci)

            # ---- layer 2 spmm ----
            if L2_DR:
                spmm_merged_l2_dr(sink_out)
                spmm_pass_dr(2, 1, sink_out, n_po=2)
            else:
                spmm_merged_l2(sink_out)
                spmm_pass_l2(1, sink_out)

    nc.compile()
    return nc


def _get_nc():
    if "nc" not in _CACHE:
        _CACHE["nc"] = _build_bass()
    return _CACHE["nc"]


def _new_of_old():
    """old global node index -> gathered contraction index."""
    idx = np.arange(NPAD)
    k = idx // R
    r = idx % R
    return np.where(
        r < 512,
        k * 512 + r,
        np.where(
            r < 1024,
            CH_BASE[1] + k * 512 + (r - 512),
            CH_BASE[2] + k * 256 + (r - 1024),
        ),
    )


def _preprocess(x, edge_row, edge_col, edge_val, W0, W1, W2, b0, b1, b2):
    x = np.asarray(x, np.float32)
    edge_row = np.asarray(edge_row, np.int64)
    edge_col = np.asarray(edge_col, np.int64)
    edge_val = np.asarray(edge_val, np.float32)

    new_of_old = _new_of_old()

    # dense per-core adjacency blocks, transposed + permuted:
    # adjT[k][new_of_old[c], r_local] = sum of vals of edges (k*R+r_local, c)
    adjT = np.zeros((NCORES, NPAD, R), np.float32)
    core = edge_row // R
    r_local = edge_row % R
    np.add.at(adjT, (core, new_of_old[edge_col], r_local), edge_val)

    # flat blocks [20, P, KSUB, R] in contraction order C_ORDER
    def blocks_of(a):  # a: [NPAD, R] for one core
        out = []
        for c in C_ORDER:
            base, _, kt, groups = CH_BASE[c], *CHUNKS[c][1:]
            ac = a[base : base + kt * P].reshape(kt, P, R)
            for g in range(groups):
                out.append(ac[g * KSUB : (g + 1) * KSUB].transpose(1, 0, 2))
        return np.stack(out)  # [20, P, KSUB, R]

    adjA01, adjA2, adjB01, adjB2 = [], [], [], []
    for k in range(NCORES):
        blk = blocks_of(adjT[k])
        a4 = blk.astype(float8_e4m3)
        adjA01.append(np.ascontiguousarray(
            np.stack([a4[..., 0:512], a4[..., 512:1024]], axis=0)))
        adjA2.append(np.ascontiguousarray(a4[..., 1024:1280]))
        if not L2_DR:
            a3 = blk.astype(float8_e3m4)
            adjB01.append(np.ascontiguousarray(
                np.stack([a3[..., 0:512], a3[..., 512:1024]], axis=0)))
            adjB2.append(np.ascontiguousarray(a3[..., 1024:1280]))
    del adjT

    # x in gathered node-major fp8 blocks [20, P, KSUB, 512] (replicated)
    x_pad = np.zeros((NPAD, x.shape[1]), np.float32)
    x_pad[:N] = x
    old_of_new = np.empty(NPAD, np.int64)
    old_of_new[new_of_old] = np.arange(NPAD)
    x_new = x_pad[old_of_new]  # rows in gathered order
    xch_blocks = []
    for c in C_ORDER:
        base, kt = CH_BASE[c], CHUNKS[c][2]
        seg = x_new[base : base + kt * P].reshape(kt, P, x.shape[1])
        for g in range(CHUNKS[c][3]):
            xch_blocks.append(
                seg[g * KSUB : (g + 1) * KSUB].transpose(1, 0, 2)
            )
    xch = np.ascontiguousarray(np.stack(xch_blocks)).astype(float8_e4m3)

    in_maps = []
    for k in range(NCORES):
        im = {
            "xch": xch,
            "adjA01": adjA01[k],
            "adjA2": adjA2[k],
            "W0": (np.asarray(W0, np.float32) * 16.0).astype(float8_e4m3),
            "W1": np.asarray(W1, np.float32).astype(bfloat16),
            "W2": np.asarray(W2, np.float32).astype(bfloat16),
            "b0": np.asarray(b0, np.float32).reshape(-1, 1),
            "b1": np.asarray(b1, np.float32).reshape(-1, 1),
            "b2": np.asarray(b2, np.float32).reshape(-1, 1),
        }
        if not L2_DR:
            im["adjB01"] = adjB01[k]
            im["adjB2"] = adjB2[k]
        in_maps.append(im)
    return in_maps


def kernel(x, edge_row, edge_col, edge_val, W0, W1, W2, b0, b1, b2):
    global LAST_RESULT
    from concourse.bass_utils import run_bass_kernel_spmd

    nc = _get_nc()
    in_maps = _preprocess(
        x, edge_row, edge_col, edge_val, W0, W1, W2, b0, b1, b2
    )
    res = run_bass_kernel_spmd(
        nc,
        in_maps,
        core_ids=list(range(NCORES)),
        trace=bool(int(os.environ.get("GCN_TRACE", "0"))),
    )
    LAST_RESULT = res

    outT = np.concatenate(
        [np.asarray(res.results[k]["outT"]) for k in range(NCORES)], axis=1
    )  # [256, 10240]
    return np.ascontiguousarray(outT.T[:N]).astype(np.float32)


# revision 10
# speedup vs baseline: 1.2060x; 1.2060x over previous
"""GCN (3-layer, skip-concat) on 8 Trainium2 NeuronCores.

Strategy (hardcoded for N=10000, E=320000, dims 512/512/256):
  - Row-partition nodes across 8 cores (1280 padded rows each, N padded
    to 10240). The adjacency shard is densified on the host into
    A_k^T [10240, 1280] fp8 (e4m3 for layers 0/1 DoubleRow, e3m4 for
    the final layer's plain-rate SpMM).
  - Layer 0 runs SPMM-FIRST: since A@(x@W0) = (A@x)@W0 and x is
    replicated on every core, the kernel skips the full dense x@W0
    entirely. x is staged host-side in the gathered node-major fp8
    layout (same layout the SpMM stationary wants), the L0 SpMM
    computes Z^T = (A_k @ x)^T feature-major, and a tiny dense
    (W0 stationary, 0.34 GMAC) produces Y0^T = W0^T Z^T. No layer-0
    collective at all; the CC-stream bootstrap (~60-75us) hides behind
    the L0 SpMM.
  - Layers 0/1 SpMM: S blocks (e4m3) stationary + A^T (e4m3) moving with
    perf_mode=DoubleRow -> 256-deep contraction per instruction. W0 is
    host-scaled by 16 (clears the e4m3 denormal floor); the Y0 sink
    applies 1/16. Layer 2 SpMM: e3m4 S (host-scale 1/8, sink x8) x e3m4
    A, plain matmuls (DR requires e4m3 and would roughly double the
    output error). End-to-end rel err ~1.1e-2 (gate 2e-2).
  - Activations live feature-major (X^T) in SBUF. Per layer:
      S_k   = X^T.T @ W          (node-major [1280, d_out], PE)
      S     = AllGather(S_k)     (fp8, HBM collective, 3 chunks)
      Y^T   = S_tiles^T @ A_k^T  (feature-major, PE; S tiles stationary)
      X' ^T = [relu(Y^T + b); (Y^T + b)]   (partition-axis concat, free)
  - Chunking: 3 gathered chunks per layer = local row ranges
    [0,512) / [512,1024) / [1024,1280), produced by passes nci0/1/2.
  - A tiny warm-up AllGather at program start absorbs the CC-stream
    bootstrap (~35-110us) + initial cross-core skew, so the first real
    gather never eats them.
  - Queue discipline: sync = x_ch blocks + adjacency prefetch + deferred
    W0/W1/W2/bias loads (trickled); scalar = epilogue drains (alternating
    with vector) + bounce writes + output writes; VECTOR ring = gathered-
    chunk reloads (s_load) so a reload waiting on an AllGather can never
    head-of-line-block the bounce writes that trigger the NEXT AllGather;
    gpsimd = collective triggers.
  - SpMM PSUM tiles rotate through 5 tags so the first matmul of each
    pass never waits on the previous pass's epilogue draining the bank
    (5 spmm banks + 2 dense banks = 7 of 8).
"""

import os
import numpy as np
from ml_dtypes import bfloat16, float8_e3m4, float8_e4m3

N = 10000
NPAD = 10240
NCORES = 8
R = NPAD // NCORES  # 1280 rows per core
P = 128
CT = NPAD // P  # 80 contraction tiles for the SpMM
KSUB = 4  # contraction subtiles per adjacency DMA block

D0_IN, D0_OUT = 512, 512
D1_IN, D1_OUT = 1024, 512
D2_IN, D2_OUT = 1024, 256

# n-chunks of the 1280-wide free dim (PSUM bank = 512 fp32); pass nci
# produces output chunk nci (same index).
N_CHUNKS = [(0, 512), (512, 512), (1024, 256)]
NCI_ORDER = (0, 1, 2)
# chunk c -> (local row0, nrows, gathered ktiles, KSUB-groups)
CHUNKS = {0: (0, 512, 32, 8), 1: (512, 512, 32, 8), 2: (1024, 256, 16, 4)}
C_ORDER = (0, 1, 2)  # contraction + production order
CH_BASE = {0: 0, 1: 4096, 2: 8192}  # gathered-row base of each chunk
# flat adjacency block order: (chunk, group) pairs in contraction order
BLOCKS = [(c, g) for c in C_ORDER for g in range(CHUNKS[c][3])]  # 20
# dense m-tiles whose lhsT columns come from pass nci's epilogue
M_OF_NC = {0: (0, 1, 2, 3), 1: (4, 5, 6, 7), 2: (8, 9)}

# layer-2 SpMM in e4m3 DoubleRow (faster, ~+0.006 rel err) vs e3m4 plain
L2_DR = bool(int(os.environ.get("GCN_L2DR", "0")))

_CACHE = {}
LAST_RESULT = None  # BassKernelResults of the most recent run (for test.py)


def _build_bass():
    import concourse.bass as bass
    import concourse.bacc as bacc
    import concourse.mybir as mybir
    import concourse.tile as tile

    dt = mybir.dt
    bf16 = dt.bfloat16
    e4 = dt.float8e4
    e3 = dt.float8e3
    f32 = dt.float32
    ts = bass.ts
    DR = mybir.MatmulPerfMode.DoubleRow

    nc = bacc.Bacc(
        "TRN2",
        target_bir_lowering=False,
        debug=False,
        enable_asserts=False,
        num_devices=NCORES,
    )

    # x in gathered node-major fp8 blocks (same layout as the SpMM
    # stationary chunks): [20 blocks, P, KSUB, 512]
    xch_d = nc.dram_tensor("xch", [20, P, KSUB, 512], e4, kind="ExternalInput")
    # adjacency blocks [P, KSUB, nw]: e4m3 copy (layers 0/1, DoubleRow)
    # and e3m4 copy (layer 2); nci0/1 are the 512-wide column chunks.
    adjA01_d = nc.dram_tensor("adjA01", [2, 20, P, KSUB, 512], e4, kind="ExternalInput")
    adjA2_d = nc.dram_tensor("adjA2", [20, P, KSUB, 256], e4, kind="ExternalInput")
    if not L2_DR:
        adjB01_d = nc.dram_tensor("adjB01", [2, 20, P, KSUB, 512], e3, kind="ExternalInput")
        adjB2_d = nc.dram_tensor("adjB2", [20, P, KSUB, 256], e3, kind="ExternalInput")
    W_d = [
        nc.dram_tensor("W0", [D0_IN, D0_OUT], e4, kind="ExternalInput"),
        nc.dram_tensor("W1", [D1_IN, D1_OUT], bf16, kind="ExternalInput"),
        nc.dram_tensor("W2", [D2_IN, D2_OUT], bf16, kind="ExternalInput"),
    ]
    b_d = [
        nc.dram_tensor("b0", [D0_OUT, 1], f32, kind="ExternalInput"),
        nc.dram_tensor("b1", [D1_OUT, 1], f32, kind="ExternalInput"),
        nc.dram_tensor("b2", [D2_OUT, 1], f32, kind="ExternalInput"),
    ]
    outT_d = nc.dram_tensor("outT", [D2_OUT, R], f32, kind="ExternalOutput")

    DIMS = [(D0_IN, D0_OUT), (D1_IN, D1_OUT), (D2_IN, D2_OUT)]
    # S dtype on the gather path (stationary dtype of the CONSUMING spmm)
    S2_DT = e4 if L2_DR else e3
    S_DT = [None, e4, S2_DT]
    S2_SCALE = 1.0 if L2_DR else 0.125  # host-semantics scale of S2

    with tile.TileContext(nc) as tc:
        ctx_pools = (
            tc.tile_pool(name="persist", bufs=1),
            tc.tile_pool(name="work", bufs=3),
            tc.tile_pool(name="psum", bufs=1, space="PSUM"),
            tc.tile_pool(name="dram", bufs=1, space="DRAM"),
        )
        with ctx_pools[0] as persist, ctx_pools[1] as work, \
             ctx_pools[2] as psum_pool, ctx_pools[3] as dram_pool:

            # ---- resident weights / biases (all trickled on sync) ----
            w_sb = []
            for L, (d_in, d_out) in enumerate(DIMS):
                wt = persist.tile(
                    [P, d_in // P, d_out], e4 if L == 0 else bf16,
                    name=f"w{L}", tag=f"w{L}"
                )
                w_sb.append(wt)

            b_sb = []
            for L, (d_in, d_out) in enumerate(DIMS):
                tiles = []
                for pi in range(d_out // P):
                    bt = persist.tile([P, 1], f32, name=f"b{L}_{pi}", tag=f"b{L}_{pi}")
                    tiles.append(bt)
                b_sb.append(tiles)

            def deferred_weight_dmas():
                """generator of thunks: W0/W1/W2/bias loads, trickled into
                the sync queue's spare issue slots during L0's first pass.
                Order matters: w0+b0 are needed first (~45us), W1 next
                (~55us), W2 last (~150us)."""
                for c in range(DIMS[0][0] // P):
                    yield lambda c=c: nc.sync.dma_start(
                        w_sb[0][:, c, :], W_d[0][ts(c, P), :]
                    )
                for pi in range(DIMS[0][1] // P):
                    yield lambda pi=pi: nc.sync.dma_start(
                        b_sb[0][pi][:], b_d[0][ts(pi, P), :]
                    )
                for L in (1, 2):
                    d_in = DIMS[L][0]
                    for c in range(d_in // P):
                        yield lambda L=L, c=c: nc.sync.dma_start(
                            w_sb[L][:, c, :], W_d[L][ts(c, P), :]
                        )
                for L in (1, 2):
                    for pi in range(DIMS[L][1] // P):
                        yield lambda L=L, pi=pi: nc.sync.dma_start(
                            b_sb[L][pi][:], b_d[L][ts(pi, P), :]
                        )

            # ---- activations X^T (feature-major), one 3D tile per layer ----
            xt1 = persist.tile([P, D1_IN // P, R], bf16, name="xt1", tag="xt1")
            xt2 = persist.tile([P, D2_IN // P, R], bf16, name="xt2", tag="xt2")
            xts = [None, xt1, xt2]

            # Z^T = (A_k @ x)^T feature-major, fp8 (rhs of the W0 dense)
            zt = persist.tile([P, D0_IN // P, R], e4, name="zt", tag="zt")

            # gathered stationary chunks: s_ch8 holds x (layer 0) then S1
            # (layer 1); s_ch16 holds S2 for the final layer.
            s_ch8 = {
                c: persist.tile(
                    [P, CHUNKS[c][2], 512], e4, name=f"s8_{c}", tag=f"s8_{c}"
                )
                for c in range(3)
            }
            s_ch16 = {
                c: persist.tile(
                    [P, CHUNKS[c][2], 256], S2_DT, name=f"s16_{c}", tag=f"s16_{c}"
                )
                for c in range(3)
            }
            s_ch_of = [s_ch8, s_ch8, s_ch16]

            # tiny warm-up collective: absorbs the CC-stream bootstrap
            # (~35-110us, starts ~21us in) and initial cross-core skew so
            # the first REAL all-gather doesn't eat them.
            cc_warm_in = dram_pool.tile([8, 8], bf16, name="ccw_in", tag="ccw_in")
            cc_warm_out = dram_pool.tile(
                [8 * NCORES, 8], bf16, name="ccw_out", tag="ccw_out",
                addr_space="Shared",
            )

            s_bounce = [
                None,
                dram_pool.tile([R, DIMS[1][1]], e4, name="s_bounce1", tag="sb1"),
                dram_pool.tile([R, DIMS[2][1]], S2_DT, name="s_bounce2", tag="sb2"),
            ]
            s_all = [
                None,
                {
                    c: dram_pool.tile(
                        [NCORES * CHUNKS[c][1], DIMS[1][1]], e4,
                        name=f"s_all1_{c}", tag=f"sa1_{c}", addr_space="Shared",
                    )
                    for c in range(3)
                },
                {
                    c: dram_pool.tile(
                        [NCORES * CHUNKS[c][1], DIMS[2][1]], S2_DT,
                        name=f"s_all2_{c}", tag=f"sa2_{c}", addr_space="Shared",
                    )
                    for c in range(3)
                },
            ]

            # rotating spmm psum tags: 5 tags over passes of <=4 tiles each
            sp_ctr = [0]

            def dense_m(L, m):
                """dense S_k m-tile: psum = xt.T @ W, cast to S dtype, bounce."""
                d_in, d_out = DIMS[L]
                n_ct = d_in // P
                dps = psum_pool.tile(
                    [P, d_out], f32, name=f"dps_{L}_{m}", tag="dense_ps", bufs=2
                )
                for c in range(n_ct):
                    nc.tensor.matmul(
                        dps[:],
                        lhsT=xts[L][:, c, ts(m, P)],
                        rhs=w_sb[L][:, c, :],
                        start=(c == 0),
                        stop=(c == n_ct - 1),
                    )
                s_sb = work.tile(
                    [P, d_out], S_DT[L], name=f"ssb_{L}_{m}", tag=f"s_sb{L}", bufs=4
                )
                # alternate the psum->fp8 drain between vector and
                # scalar so neither engine rate-limits the dense bursts
                sc = S2_SCALE if L == 2 else 1.0
                if m % 2 == 0:
                    if sc == 1.0:
                        nc.vector.tensor_copy(s_sb[:], dps[:])
                    else:
                        nc.vector.tensor_scalar_mul(s_sb[:], dps[:], sc)
                else:
                    nc.scalar.activation(
                        s_sb[:], dps[:], mybir.ActivationFunctionType.Copy,
                        scale=sc,
                    )
                # bounce on the sync queue: the adjacency prefetch there has
                # many buffers of slack, while the scalar ring carries the
                # s_load reloads which stall on AllGather completion (a
                # bounce stuck behind one would delay the NEXT AllGather).
                nc.sync.dma_start(s_bounce[L][ts(m, P), :], s_sb[:])

            def cc_warmup():
                nc.gpsimd.collective_compute(
                    "AllGather",
                    mybir.AluOpType.bypass,
                    replica_groups=[list(range(NCORES))],
                    ins=[cc_warm_in[:].opt()],
                    outs=[cc_warm_out[:].opt()],
                )

            def ag_issue(L, c):
                """all-gather chunk c of layer L's S (writes s_all only)."""
                r0, nrows, _, _ = CHUNKS[c]
                nc.gpsimd.collective_compute(
                    "AllGather",
                    mybir.AluOpType.bypass,
                    replica_groups=[list(range(NCORES))],
                    ins=[s_bounce[L][r0 : r0 + nrows, :].opt()],
                    outs=[s_all[L][c].opt()],
                )

            def s_load(L, c):
                """load gathered chunk c into SBUF for layer L's spmm, in
                two halves so the first k-tiles land sooner. Issued on the
                scalar ring, which carries ONLY these reloads (+ late output
                writes): a reload stuck waiting on an AllGather can then
                never block the bounce writes (sync ring) that trigger the
                next AllGather."""
                d_out = DIMS[L][1]
                src = s_all[L][c].rearrange("(t p) d -> p t d", p=P)
                dst = s_ch_of[L][c]
                kt = CHUNKS[c][2]
                q = kt // 4
                for qi in range(4):
                    nc.scalar.dma_start(
                        dst[:, qi * q : (qi + 1) * q, :d_out],
                        src[:, qi * q : (qi + 1) * q, :],
                    )

            def spmm_pass_dr(L, nci, sink, chunk_hook=None, n_po=None,
                            block_hook=None):
                """DoubleRow SpMM pass: e4m3 stationary x e4m3 A moving.

                chunk_hook(c) is emitted right after chunk c's last block in
                the contraction loop -- used on the layer's final pass to
                emit the next layer's s_ch reloads as early as the WAR
                hazard allows (ahead of this pass's sinks in queue order).
                block_hook(bi) is emitted just BEFORE block bi's adjacency
                DMA -- layer 0's first pass uses it to interleave the x_ch
                stationary loads with the adjacency stream.
                """
                if n_po is None:
                    n_po = DIMS[L][1] // P
                s_ch = s_ch_of[L]
                n0, nw = N_CHUNKS[nci]
                sp_ps = []
                for p in range(n_po):
                    tag = sp_ctr[0] % 5
                    sp_ctr[0] += 1
                    sp_ps.append(
                        psum_pool.tile(
                            [P, nw], f32, name=f"sp_{L}_{nci}_{p}", tag=f"sp{tag}"
                        )
                    )
                first = True
                for bi, (c, g) in enumerate(BLOCKS):
                    if block_hook is not None:
                        block_hook(bi)
                    if nci < 2:
                        at = work.tile(
                            [P, KSUB, 512], e4,
                            name=f"a4_{L}_{nci}_{bi}", tag="at4", bufs=12,
                        )
                        nc.sync.dma_start(at[:], adjA01_d[nci, bi])
                    else:
                        at = work.tile(
                            [P, KSUB, 256], e4,
                            name=f"a4n2_{L}_{bi}", tag="at4n2", bufs=8,
                        )
                        nc.sync.dma_start(at[:], adjA2_d[bi])
                    for sp in range(2):
                        last = (bi == len(BLOCKS) - 1 and sp == 1)
                        kk = g * KSUB + 2 * sp
                        for p in range(n_po):
                            nc.tensor.matmul(
                                sp_ps[p][:],
                                lhsT=s_ch[c][:, kk : kk + 2, ts(p, P)],
                                rhs=at[:, 2 * sp : 2 * sp + 2, :],
                                start=first,
                                stop=last,
                                perf_mode=DR,
                            )
                        first = False
                    if chunk_hook is not None and g == CHUNKS[c][3] - 1:
                        chunk_hook(c)
                for p in range(n_po):
                    sink(p, sp_ps[p], n0, nw)

            def spmm_merged_l2_dr(sink):
                """Merged nci0+nci2 DR pass for layer 2 (e4m3): each
                stationary streams 768 moving columns (512 + 256)."""
                n_po = DIMS[2][1] // P  # 2
                sp_ps = []
                for i in range(2 * n_po):
                    tag = sp_ctr[0] % 5
                    sp_ctr[0] += 1
                    sp_ps.append(
                        psum_pool.tile(
                            [P, 512], f32, name=f"l2m_{i}", tag=f"sp{tag}"
                        )
                    )
                ps0 = sp_ps[:n_po]
                ps2 = sp_ps[n_po:]
                first = True
                for bi, (c, g) in enumerate(BLOCKS):
                    at0 = work.tile(
                        [P, KSUB, 512], e4, name=f"l2a_{bi}", tag="at4", bufs=12,
                    )
                    nc.sync.dma_start(at0[:], adjA01_d[0, bi])
                    at2 = work.tile(
                        [P, KSUB, 256], e4, name=f"l2c_{bi}", tag="at4n2", bufs=8,
                    )
                    nc.sync.dma_start(at2[:], adjA2_d[bi])
                    for sp in range(2):
                        last = (bi == len(BLOCKS) - 1 and sp == 1)
                        kk = g * KSUB + 2 * sp
                        for p in range(n_po):
                            lhsT = s_ch16[c][:, kk : kk + 2, ts(p, P)]
                            nc.tensor.matmul(
                                ps0[p][:, :],
                                lhsT=lhsT,
                                rhs=at0[:, 2 * sp : 2 * sp + 2, :],
                                start=first,
                                stop=last,
                                perf_mode=DR,
                            )
                            nc.tensor.matmul(
                                ps2[p][:, :256],
                                lhsT=lhsT,
                                rhs=at2[:, 2 * sp : 2 * sp + 2, :],
                                start=first,
                                stop=last,
                                perf_mode=DR,
                            )
                        first = False
                for p in range(n_po):
                    sink(p, ps0[p], 0, 512)
                for p in range(n_po):
                    sink(p, ps2[p][:, :256], 1024, 256)

            def spmm_pass_l2(nci, sink):
                """Layer-2 SpMM pass: e3m4 S stationary x e3m4 A moving."""
                n_po = DIMS[2][1] // P
                n0, nw = N_CHUNKS[nci]
                sp_ps = []
                for p in range(n_po):
                    tag = sp_ctr[0] % 5
                    sp_ctr[0] += 1
                    sp_ps.append(
                        psum_pool.tile(
                            [P, nw], f32, name=f"sp_2_{nci}_{p}", tag=f"sp{tag}"
                        )
                    )
                first = True
                for bi, (c, g) in enumerate(BLOCKS):
                    if nci < 2:
                        at = work.tile(
                            [P, KSUB, 512], e3,
                            name=f"a3_{nci}_{bi}", tag="at3", bufs=10,
                        )
                        nc.sync.dma_start(at[:], adjB01_d[nci, bi])
                    else:
                        at = work.tile(
                            [P, KSUB, 256], e3,
                            name=f"a3n2_{bi}", tag="at3n2", bufs=5,
                        )
                        nc.sync.dma_start(at[:], adjB2_d[bi])
                    for s in range(KSUB):
                        last = (bi == len(BLOCKS) - 1 and s == KSUB - 1)
                        for p in range(n_po):
                            nc.tensor.matmul(
                                sp_ps[p][:],
                                lhsT=s_ch16[c][:, g * KSUB + s, ts(p, P)],
                                rhs=at[:, s, :],
                                start=first,
                                stop=last,
                            )
                        first = False
                for p in range(n_po):
                    sink(p, sp_ps[p], n0, nw)

            def spmm_merged_l2(sink):
                """Merged nci0+nci2 pass for layer 2 (plain e3m4 matmuls):
                each stationary LDW streams 768 columns (512 + 256), hiding
                the LDWEIGHTS that otherwise dominate the narrow nci2 pass.
                Each 256-wide output gets its OWN full PSUM bank."""
                n_po = DIMS[2][1] // P  # 2
                sp_ps = []
                for i in range(2 * n_po):
                    tag = sp_ctr[0] % 5
                    sp_ctr[0] += 1
                    sp_ps.append(
                        psum_pool.tile(
                            [P, 512], f32, name=f"l2m_{i}", tag=f"sp{tag}"
                        )
                    )
                ps0 = sp_ps[:n_po]
                ps2 = sp_ps[n_po:]
                first = True
                for bi, (c, g) in enumerate(BLOCKS):
                    at0 = work.tile(
                        [P, KSUB, 512], e3, name=f"l2a_{bi}", tag="at3", bufs=10,
                    )
                    nc.sync.dma_start(at0[:], adjB01_d[0, bi])
                    at2 = work.tile(
                        [P, KSUB, 256], e3, name=f"l2c_{bi}", tag="at3n2", bufs=8,
                    )
                    nc.sync.dma_start(at2[:], adjB2_d[bi])
                    for s in range(KSUB):
                        last = (bi == len(BLOCKS) - 1 and s == KSUB - 1)
                        for p in range(n_po):
                            lhsT = s_ch16[c][:, g * KSUB + s, ts(p, P)]
                            nc.tensor.matmul(
                                ps0[p][:, :],
                                lhsT=lhsT,
                                rhs=at0[:, s, :],
                                start=first,
                                stop=last,
                            )
                            nc.tensor.matmul(
                                ps2[p][:, :256],
                                lhsT=lhsT,
                                rhs=at2[:, s, :],
                                start=first,
                                stop=last,
                            )
                        first = False
                for p in range(n_po):
                    sink(p, ps0[p], 0, 512)
                for p in range(n_po):
                    sink(p, ps2[p][:, :256], 1024, 256)

            def sink_z(p, ps, n0, nw):
                """drain Z psum -> zt fp8 (no scale), alternating engines."""
                if p % 2 == 0:
                    nc.vector.tensor_copy(zt[:, p, n0 : n0 + nw], ps[:])
                else:
                    nc.scalar.activation(
                        zt[:, p, n0 : n0 + nw], ps[:],
                        mybir.ActivationFunctionType.Copy,
                    )

            def w0_dense(nci):
                """Y0^T chunk = W0^T @ Z^T chunk (DR, W0 stationary),
                sinking straight into xt1 (relu+bias / +bias concat)."""
                n0, nw = N_CHUNKS[nci]
                n_po = DIMS[0][1] // P  # 4
                n_ct = DIMS[0][0] // P  # 4
                for po in range(n_po):
                    dps = psum_pool.tile(
                        [P, nw], f32, name=f"w0d_{nci}_{po}", tag="dense_ps",
                        bufs=2,
                    )
                    for sp in range(n_ct // 2):
                        nc.tensor.matmul(
                            dps[:],
                            lhsT=w_sb[0][:, 2 * sp : 2 * sp + 2, ts(po, P)],
                            rhs=zt[:, 2 * sp : 2 * sp + 2, n0 : n0 + nw],
                            start=(sp == 0),
                            stop=(sp == n_ct // 2 - 1),
                            perf_mode=DR,
                        )
                    # psum holds 16*Y0 (W0 host-scaled); sink applies 1/16
                    nc.scalar.activation(
                        xt1[:, po, n0 : n0 + nw],
                        dps[:],
                        mybir.ActivationFunctionType.Relu,
                        bias=b_sb[0][po][:],
                        scale=1.0 / 16.0,
                    )
                    nc.vector.tensor_scalar(
                        xt1[:, n_po + po, n0 : n0 + nw],
                        dps[:], 1.0 / 16.0, b_sb[0][po][:],
                        mybir.AluOpType.mult, mybir.AluOpType.add,
                    )

            def sink_mid(L):
                n_po = DIMS[L][1] // P

                def sink(p, ps, n0, nw):
                    nc.scalar.activation(
                        xts[L + 1][:, p, n0 : n0 + nw],
                        ps[:],
                        mybir.ActivationFunctionType.Relu,
                        bias=b_sb[L][p][:],
                    )
                    nc.vector.tensor_scalar_add(
                        xts[L + 1][:, n_po + p, n0 : n0 + nw],
                        ps[:],
                        b_sb[L][p][:],
                    )

                return sink

            def sink_out(p, ps, n0, nw):
                ot = work.tile([P, nw], f32, name=f"ot_{n0}_{p}", tag="ot", bufs=3)
                nc.vector.tensor_scalar(
                    ot[:], ps[:, :], 1.0 / S2_SCALE, b_sb[2][p][:],
                    mybir.AluOpType.mult, mybir.AluOpType.add,
                )
                nc.scalar.dma_start(outT_d[ts(p, P), n0 : n0 + nw], ot[:])

            # ================= pipeline =================
            cc_warmup()

            wgen = deferred_weight_dmas()

            def xch_block_hook(bi):
                """interleave x_ch stationary loads with pass 0's adjacency
                stream, keeping the x_ch stream 4 blocks AHEAD of the
                adjacency stream (the stationary is the fresh dependency),
                then trickle the resident weight loads once the pipeline
                is primed."""
                if bi == 0:
                    for bj in range(4):
                        c, g = BLOCKS[bj]
                        nc.sync.dma_start(
                            s_ch8[c][:, g * KSUB : (g + 1) * KSUB, :],
                            xch_d[bj],
                        )
                bj = bi + 4
                if bj < len(BLOCKS):
                    c, g = BLOCKS[bj]
                    nc.sync.dma_start(
                        s_ch8[c][:, g * KSUB : (g + 1) * KSUB, :], xch_d[bj]
                    )
                if bi >= 10:
                    for _ in range(2):
                        th = next(wgen, None)
                        if th is not None:
                            th()

            # ---- layer 0: spmm-first ----
            for nci in NCI_ORDER:
                hook = None
                if nci == NCI_ORDER[-1]:
                    # L1's s_ch8 reloads, emitted chunk-by-chunk as the
                    # WAR hazard (this pass's chunk-c reads) clears.
                    hook = lambda c: s_load(1, c) if c != 2 else None
                spmm_pass_dr(
                    0, nci, sink_z, chunk_hook=hook,
                    block_hook=xch_block_hook if nci == 0 else None,
                )
                # trickle remaining weight/bias loads between passes
                for _ in range(8):
                    th = next(wgen, None)
                    if th is not None:
                        th()
                w0_dense(nci)
                for m in M_OF_NC[nci]:
                    dense_m(1, m)
                ag_issue(1, nci)
                if nci == NCI_ORDER[-1]:
                    s_load(1, 2)

            # ---- layer 1 spmm + layer 2 dense + S2 gathers ----
            for nci in NCI_ORDER:
                spmm_pass_dr(1, nci, sink_mid(1))
                for m in M_OF_NC[nci]:
                    dense_m(2, m)
                ag_issue(2, nci)
                # s_ch16 is untouched before layer 2: load each gathered
                # chunk as soon as its AG is issued (scalar ring, so it
                # can never block the sync-ring bounce writes).
                s_load(2, nci)

            # ---- layer 2 spmm ----
            if L2_DR:
                spmm_merged_l2_dr(sink_out)
                spmm_pass_dr(2, 1, sink_out, n_po=2)
            else:
                spmm_merged_l2(sink_out)
                spmm_pass_l2(1, sink_out)

    nc.compile()
    return nc


def _get_nc():
    if "nc" not in _CACHE:
        _CACHE["nc"] = _build_bass()
    return _CACHE["nc"]


def _new_of_old():
    """old global node index -> gathered contraction index."""
    idx = np.arange(NPAD)
    k = idx // R
    r = idx % R
    return np.where(
        r < 512,
        k * 512 + r,
        np.where(
            r < 1024,
            CH_BASE[1] + k * 512 + (r - 512),
            CH_BASE[2] + k * 256 + (r - 1024),
        ),
    )


def _preprocess(x, edge_row, edge_col, edge_val, W0, W1, W2, b0, b1, b2):
    x = np.asarray(x, np.float32)
    edge_row = np.asarray(edge_row, np.int64)
    edge_col = np.asarray(edge_col, np.int64)
    edge_val = np.asarray(edge_val, np.float32)

    new_of_old = _new_of_old()

    # dense per-core adjacency blocks, transposed + permuted:
    # adjT[k][new_of_old[c], r_local] = sum of vals of edges (k*R+r_local, c)
    adjT = np.zeros((NCORES, NPAD, R), np.float32)
    core = edge_row // R
    r_local = edge_row % R
    np.add.at(adjT, (core, new_of_old[edge_col], r_local), edge_val)

    # flat blocks [20, P, KSUB, R] in contraction order C_ORDER
    def blocks_of(a):  # a: [NPAD, R] for one core
        out = []
        for c in C_ORDER:
            base, _, kt, groups = CH_BASE[c], *CHUNKS[c][1:]
            ac = a[base : base + kt * P].reshape(kt, P, R)
            for g in range(groups):
                out.append(ac[g * KSUB : (g + 1) * KSUB].transpose(1, 0, 2))
        return np.stack(out)  # [20, P, KSUB, R]

    adjA01, adjA2, adjB01, adjB2 = [], [], [], []
    for k in range(NCORES):
        blk = blocks_of(adjT[k])
        a4 = blk.astype(float8_e4m3)
        adjA01.append(np.ascontiguousarray(
            np.stack([a4[..., 0:512], a4[..., 512:1024]], axis=0)))
        adjA2.append(np.ascontiguousarray(a4[..., 1024:1280]))
        if not L2_DR:
            a3 = blk.astype(float8_e3m4)
            adjB01.append(np.ascontiguousarray(
                np.stack([a3[..., 0:512], a3[..., 512:1024]], axis=0)))
            adjB2.append(np.ascontiguousarray(a3[..., 1024:1280]))
    del adjT

    # x in gathered node-major fp8 blocks [20, P, KSUB, 512] (replicated)
    x_pad = np.zeros((NPAD, x.shape[1]), np.float32)
    x_pad[:N] = x
    old_of_new = np.empty(NPAD, np.int64)
    old_of_new[new_of_old] = np.arange(NPAD)
    x_new = x_pad[old_of_new]  # rows in gathered order
    xch_blocks = []
    for c in C_ORDER:
        base, kt = CH_BASE[c], CHUNKS[c][2]
        seg = x_new[base : base + kt * P].reshape(kt, P, x.shape[1])
        for g in range(CHUNKS[c][3]):
            xch_blocks.append(
                seg[g * KSUB : (g + 1) * KSUB].transpose(1, 0, 2)
            )
    xch = np.ascontiguousarray(np.stack(xch_blocks)).astype(float8_e4m3)

    in_maps = []
    for k in range(NCORES):
        im = {
            "xch": xch,
            "adjA01": adjA01[k],
            "adjA2": adjA2[k],
            "W0": (np.asarray(W0, np.float32) * 16.0).astype(float8_e4m3),
            "W1": np.asarray(W1, np.float32).astype(bfloat16),
            "W2": np.asarray(W2, np.float32).astype(bfloat16),
            "b0": np.asarray(b0, np.float32).reshape(-1, 1),
            "b1": np.asarray(b1, np.float32).reshape(-1, 1),
            "b2": np.asarray(b2, np.float32).reshape(-1, 1),
        }
        if not L2_DR:
            im["adjB01"] = adjB01[k]
            im["adjB2"] = adjB2[k]
        in_maps.append(im)
    return in_maps


def kernel(x, edge_row, edge_col, edge_val, W0, W1, W2, b0, b1, b2):
    global LAST_RESULT
    from concourse.bass_utils import run_bass_kernel_spmd

    nc = _get_nc()
    in_maps = _preprocess(
        x, edge_row, edge_col, edge_val, W0, W1, W2, b0, b1, b2
    )
    res = run_bass_kernel_spmd(
        nc,
        in_maps,
        core_ids=list(range(NCORES)),
        trace=bool(int(os.environ.get("GCN_TRACE", "0"))),
    )
    LAST_RESULT = res

    outT = np.concatenate(
        [np.asarray(res.results[k]["outT"]) for k in range(NCORES)], axis=1
    )  # [256, 10240]
    return np.ascontiguousarray(outT.T[:N]).astype(np.float32)
